# revision 23
# baseline (speedup 1.0000x reference)
"""Trainium2 Bass kernel for nn_CrossAttentionSameFrame.

Math: with the same-frame mask, each query attends to exactly one key, so
softmax weight == 1 and the attention output is just the v-projection of the
query's own context frame, broadcast over the frame's tokens:

    v[b, m, :] = context[b, m] @ Wkv[:, D:2D] + bkv[D:2D]      (k, q unused)
    y[b, m, :] = v[b, m] @ Wo + bo
    out[b, m*tpf + t, :] = y[b, m]        for t in [0, tpf)

x / Wq / bq / the k-half of Wkv are mathematically dead. The kernel is
memory-bound: the 128 MiB output write dominates.

Sharding: all 8 cores compute the tiny Y = (ctx @ Wv + bv) @ Wo + bo
(128 rows x 1024) redundantly (~9 MiB of loads, ~28 us of fp32 PE), and
each core writes 1/8 of the output: token-slots [i*32, (i+1)*32) of every
frame. With frames on partitions, the natural matmul output tile Y
[128, 1024] is stored via broadcast-source DMAs (step-0 middle dim) — no
on-chip replication at all.

Overlap structure:
  - Loads stream on the SP HWDGE ring in critical-path order: ctxT, Wv in
    eight column-chunks (PE starts after the first chunk), biases, Wo in
    four column-quarters.
  - PE warms up its p-state on dummy matmuls (memset scratch) while the
    first Wv chunk loads; V^T chunk c needs only Wv columns
    [c*128,(c+1)*128), so step-1 pipelines under the Wv load stream.
  - Y is produced in four 256-column quarters (one PSUM bank each); each
    quarter's stores go out on the ACT HWDGE ring as soon as the quarter
    lands in SBUF, overlapping the tail of the load stream.
  - bv rides the V^T PSUM->SBUF copies on DVE (per-partition scalar add);
    bo is folded into each Y matmul group as a K=1 ones-row matmul.
"""

from contextlib import ExitStack

import numpy as np

# Problem shape (hardcoded per contest rules; kernel.py must be self-contained)
B, Lq, D = 2, 16384, 1024
M = 64                  # context frames
TPF = Lq // M           # tokens per frame = 256
F = B * M               # 128 frame-rows = one full partition dim
N_CORES = 8
TPC = TPF // N_CORES    # 32 token-slots written per core
KC = D // 128           # 8 contraction chunks
REP = 8                 # broadcast reps per store DMA (>=16 crashes exec unit)
NQ = 4                  # Y column-quarters
QW = D // NQ            # 256 columns per quarter
N_WARM = 6              # PE p-state warmup matmuls

_CACHE = {}


def _build_nc():
    import concourse.bass as bass
    import concourse.mybir as mybir

    f32 = mybir.dt.float32
    nc = bass.Bass()

    # DRAM I/O (per-core views; all cores receive identical inputs)
    ctxT = nc.dram_tensor("ctxT", [D, F], f32, kind="ExternalInput")
    wvq = nc.dram_tensor("wvq", [KC, D, 128], f32, kind="ExternalInput")
    woq = nc.dram_tensor("woq", [NQ, D, QW], f32, kind="ExternalInput")
    bvr = nc.dram_tensor("bvr", [128, KC], f32, kind="ExternalInput")
    bo_i = nc.dram_tensor("bo_i", [1, D], f32, kind="ExternalInput")
    ones_i = nc.dram_tensor("ones_i", [1, 128], f32, kind="ExternalInput")
    out = nc.dram_tensor("out", [F, TPC, D], f32, kind="ExternalOutput")

    with ExitStack() as ctx:
        # SBUF working set
        ctxt_t = ctx.enter_context(nc.sbuf_tensor([128, KC, F], f32))
        wv_t = ctx.enter_context(nc.sbuf_tensor([128, KC, D], f32))
        wo_t = ctx.enter_context(nc.sbuf_tensor([128, KC, D], f32))
        bvr_t = ctx.enter_context(nc.sbuf_tensor([128, KC], f32))
        bo_t = ctx.enter_context(nc.sbuf_tensor([1, D], f32))
        ones_t = ctx.enter_context(nc.sbuf_tensor([1, 128], f32))
        vt_t = ctx.enter_context(nc.sbuf_tensor([128, KC, F], f32))
        y_t = ctx.enter_context(nc.sbuf_tensor([128, D], f32))
        scr_t = ctx.enter_context(nc.sbuf_tensor([128, QW], f32))
        # PSUM: one bank per in-flight V^T chunk and per Y quarter (PE-write
        # + DVE-read of the same bank is a fatal HW conflict). 4 + 4 banks.
        vt_ps0 = ctx.enter_context(nc.psum_tensor([128, F], f32))
        vt_ps1 = ctx.enter_context(nc.psum_tensor([128, F], f32))
        vt_ps2 = ctx.enter_context(nc.psum_tensor([128, F], f32))
        vt_ps3 = ctx.enter_context(nc.psum_tensor([128, F], f32))
        y_ps0 = ctx.enter_context(nc.psum_tensor([128, QW], f32))
        y_ps1 = ctx.enter_context(nc.psum_tensor([128, QW], f32))
        y_ps2 = ctx.enter_context(nc.psum_tensor([128, QW], f32))
        y_ps3 = ctx.enter_context(nc.psum_tensor([128, QW], f32))

        ld_ctx = ctx.enter_context(nc.semaphore())   # ctxT
        ld_wv = [
            ctx.enter_context(nc.semaphore(f"ld_wv{q}")) for q in range(KC)
        ]                                            # Wv column-eighths
        ld_pre = ctx.enter_context(nc.semaphore())   # bvr + bo
        ld_wo = [
            ctx.enter_context(nc.semaphore(f"ld_wo{q}")) for q in range(NQ)
        ]                                            # Wo column-quarters
        sem_w = ctx.enter_context(nc.semaphore())    # warmup scratch memset
        pe1 = ctx.enter_context(nc.semaphore())      # V^T matmul groups done
        cpv = ctx.enter_context(nc.semaphore())      # V^T psum->sbuf (+bv) done
        pe2 = ctx.enter_context(nc.semaphore())      # Y quarter groups done
        cpy = ctx.enter_context(nc.semaphore())      # Y psum->sbuf (+bo) done
        st = ctx.enter_context(nc.semaphore())       # output stores done
        block = ctx.enter_context(nc.Block())

        vt_ps = [vt_ps0, vt_ps1, vt_ps2, vt_ps3]
        y_ps = [y_ps0, y_ps1, y_ps2, y_ps3]

        @block.gpsimd
        def _(gpsimd):
            gpsimd.memset(scr_t[:], 0.0).then_inc(sem_w, 1)

        @block.sync
        def _(sync):
            # Loads on the SP ring, critical-path order.
            sync.dma_start(
                ctxt_t[:], ctxT[:].rearrange("(k p) r -> p k r", p=128)
            ).then_inc(ld_ctx, 16)
            # Wv column-eighth q -> wv_t[:, :, q*128:(q+1)*128]
            for q in range(KC):
                sync.dma_start(
                    wv_t[:, :, q * 128 : (q + 1) * 128],
                    wvq[q].rearrange("(k p) n -> p k n", p=128),
                ).then_inc(ld_wv[q], 16)
                if q == 0:
                    sync.dma_start(bvr_t[:], bvr[:]).then_inc(ld_pre, 16)
                    sync.dma_start(bo_t[:], bo_i[:]).then_inc(ld_pre, 16)
                    sync.dma_start(ones_t[:], ones_i[:]).then_inc(ld_pre, 16)
            for q in range(NQ):
                sync.dma_start(
                    wo_t[:, :, q * QW : (q + 1) * QW],
                    woq[q].rearrange("(k p) n -> p k n", p=128),
                ).then_inc(ld_wo[q], 16)

        @block.tensor
        def _(tensor):
            # p-state warmup on scratch zeros while Wv quarter 0 loads
            tensor.wait_ge(sem_w, 1)
            for w in range(N_WARM):
                nc.tensor.matmul(
                    y_ps[0][:], scr_t[:, :128], scr_t[:], start=True, stop=True
                )
            # Step 1: V^T chunks.  VT_c[j, r] = sum_d Wv[d, c*128+j] * ctx[r, d]
            tensor.wait_ge(ld_ctx, 16)
            for c in range(KC):
                tensor.wait_ge(ld_wv[c], 16)
                if c >= 4:
                    # bank reuse: wait until DVE copied chunk c-4 out
                    tensor.wait_ge(cpv, c - 3)
                for k in range(KC):
                    mm = nc.tensor.matmul(
                        vt_ps[c % 4][:],
                        wv_t[:, k, c * 128 : (c + 1) * 128],
                        ctxt_t[:, k, :],
                        start=(k == 0),
                        stop=(k == KC - 1),
                    )
                    if k == KC - 1:
                        mm.then_inc(pe1, 1)
            # Step 2: Y quarters.  Y[r, n] = sum_j V[r, j] * Wo[j, n] + bo[n]
            tensor.wait_ge(cpv, KC)
            tensor.wait_ge(ld_pre, 48)
            for q in range(NQ):
                tensor.wait_ge(ld_wo[q], 16)
                ns = slice(q * QW, (q + 1) * QW)
                for c in range(KC):
                    nc.tensor.matmul(
                        y_ps[q][:],
                        vt_t[:, c, :],
                        wo_t[:, c, ns],
                        start=(c == 0),
                        stop=False,
                    )
                mm = nc.tensor.matmul(
                    y_ps[q][:], ones_t[:1, :], bo_t[:1, ns],
                    start=False, stop=True,
                )
                mm.then_inc(pe2, 1)

        @block.vector
        def _(vector):
            vector.wait_ge(ld_pre, 16)
            # V^T psum -> sbuf with per-partition bias bv
            for c in range(KC):
                vector.wait_ge(pe1, c + 1)
                nc.vector.tensor_scalar_add(
                    vt_t[:, c, :], vt_ps[c % 4][:], bvr_t[:, c : c + 1]
                ).then_inc(cpv, 1)
            # Y psum -> sbuf (bo already folded into the matmul group)
            for q in range(NQ):
                vector.wait_ge(pe2, q + 1)
                ns = slice(q * QW, (q + 1) * QW)
                nc.vector.tensor_copy(
                    y_t[:, ns], y_ps[q][:]
                ).then_inc(cpy, 1)

        @block.scalar
        def _(scalar):
            # Stores on the ACT ring: column-quarter q as soon as its Y
            # quarter is in SBUF. Broadcast-source (step-0) DMAs.
            n_st = TPC // REP
            for q in range(NQ):
                scalar.wait_ge(cpy, q + 1)
                ns = slice(q * QW, (q + 1) * QW)
                src = y_t[:, ns].unsqueeze(1).broadcast_to((F, REP, QW))
                for j in range(n_st):
                    scalar.dma_start(
                        out[:, j * REP : (j + 1) * REP, ns], src
                    ).then_inc(st, 16)
            scalar.wait_ge(st, 16 * n_st * NQ)

    return nc


def _prep_inputs(context, Wkv, bkv, Wo, bo):
    ctx_flat = np.ascontiguousarray(np.asarray(context, np.float32)).reshape(F, D)
    Wkv = np.asarray(Wkv, np.float32)
    bkv = np.asarray(bkv, np.float32)
    Wo = np.asarray(Wo, np.float32)
    wv = Wkv[:, D : 2 * D]                                       # [D, D]
    return {
        "ctxT": np.ascontiguousarray(ctx_flat.T),                # [D, F]
        "wvq": np.ascontiguousarray(wv.reshape(D, KC, 128).transpose(1, 0, 2)),
        "woq": np.ascontiguousarray(Wo.reshape(D, NQ, QW).transpose(1, 0, 2)),
        "bvr": np.ascontiguousarray(bkv[D:].reshape(KC, 128).T), # [128, KC]
        "bo_i": np.ascontiguousarray(np.asarray(bo, np.float32).reshape(1, D)),
        "ones_i": np.ones((1, 128), np.float32),
    }


def _get_nc():
    if "nc" not in _CACHE:
        _CACHE["nc"] = _build_nc()
    return _CACHE["nc"]


def run_spmd(in_map, **kwargs):
    """Run the SPMD kernel; returns BassKernelResults (test harness hook)."""
    from concourse.bass_utils import run_bass_kernel_spmd

    nc = _get_nc()
    return run_bass_kernel_spmd(
        nc, [in_map] * N_CORES, list(range(N_CORES)), **kwargs
    )


def kernel(x, context, Wq, bq, Wkv, bkv, Wo, bo):
    # x, Wq, bq and the k-half of Wkv/bkv are mathematically unused.
    in_map = _prep_inputs(context, Wkv, bkv, Wo, bo)
    res = None
    for attempt in range(3):
        try:
            res = run_spmd(in_map)
            break
        except Exception:
            # Device execution occasionally flakes (NRT_EXEC_UNIT_UNRECOVERABLE);
            # a clean retry on the same NEFF consistently succeeds.
            if attempt == 2:
                raise
            try:
                import time

                import jax

                jax.clear_caches()
                time.sleep(2.0)
            except Exception:
                pass
    assert res is not None
    O = np.empty((B, M, TPF, D), np.float32)
    for i in range(N_CORES):
        O[:, :, i * TPC : (i + 1) * TPC, :] = res.results[i]["out"].reshape(
            B, M, TPC, D
        )
    return O.reshape(B, Lq, D)


if __name__ == "__main__":
    rng = np.random.default_rng(0)
    inputs = {
        "x": rng.standard_normal((B, Lq, D), dtype=np.float32),
        "context": rng.standard_normal((B, M, D), dtype=np.float32),
        "Wq": rng.standard_normal((D, D), dtype=np.float32),
        "bq": np.zeros((D,), np.float32),
        "Wkv": rng.standard_normal((D, 2 * D), dtype=np.float32) * (D**-0.5),
        "bkv": rng.standard_normal((2 * D,), dtype=np.float32),
        "Wo": rng.standard_normal((D, D), dtype=np.float32) * (D**-0.5),
        "bo": rng.standard_normal((D,), dtype=np.float32),
    }
    out = kernel(**inputs)
    v = inputs["context"] @ inputs["Wkv"][:, D:] + inputs["bkv"][D:]
    y = v @ inputs["Wo"] + inputs["bo"]
    exp = np.repeat(y, TPF, axis=1)
    err = np.abs(out - exp).max() / np.abs(exp).max()
    print("rel err:", err)
